# revision 44
# baseline (speedup 1.0000x reference)
"""MoE experts (32 experts, top-2, SwiGLU MLP) on 8 trn2 NeuronCores.

Expert-parallel: core c owns 4 experts (sorted round-robin deal, which is
provably optimal for the sum of per-position pads). Routing metadata is
computed on host; each core gets its experts' weights pre-transposed to
matmul layout plus dispatched token activations, runs the grouped SwiGLU
MLP + routing-weight scaling on device (fp16 operands, fp32 accumulation),
and returns per-slot outputs in fp16. Host scatters/combines.

Schedule (arrived at via trace analysis; key hardware facts measured on
the way: the two HWDGE rings share one ~360 GB/s HBM port and run ~400
GB/s on 8-16KB/partition transfers but only ~150-250 on small ones; the
tile scheduler hoists dep-free DMA issues to t=0; the PE p-state ramps
0.65 -> 2.4 GHz over ~3us of continuous busy and the whole matmul stream
runs ~2.25-2.3 cols/ns once ramped):
- The ENTIRE weight stream goes on the SP ring in exact consumption
  order (FIFO = schedule; no cross-ring arbitration surprises):
  [xd0|w1(e0) it0-1] fused into one large-elem first transfer, w1(e0)
  it2-7 in 2048-col pieces, w2(e0), xd(e1-3), then per expert j:
  w1(e_j) halves, w2(e_j). The ACT ring carries only the output stores
  (dep-bound, late). Tile-pool rotation (bufs=2) staggers e3's w1
  behind e1's last use.
- 16 warm-up matmuls on a zeroed tile ramp the PE p-state while the
  first transfer is in flight, ending right as real data lands.
- mm1 is emitted it-major so it consumes w1 pieces in arrival order;
  host lays w1 out it-major (per it: 4 h-chunks of gate|up columns
  contiguous) and w2 ht2-major so mm2 slices are contiguous.
- Output tiles + stores are fp16; the last expert's store is split
  per h-chunk (own tile each, so the store depends only on its own
  scale-mul); other stores are deferred one expert so a store issue
  never delays a silu.
"""

import sys
import types

import numpy as np

# Model dims (hardcoded per problem spec nn_MoEExperts_27109833572673)
T, TOPK, E, H, I = 4096, 2, 32, 512, 1024
CAP = 2 * (T * TOPK) // E  # 512
NCORES = 8
EPC = E // NCORES  # experts per core = 4
HT = H // 128  # 4 h-tiles
IT = I // 128  # 8 i-tiles

LAST_RESULTS = None  # BassKernelResults of the most recent device run


def _ensure_profile_hook():
    """Register the NTFF profile hook if the env lacks antenv.axon_hooks.

    Only needed when tracing (BASS_TRACE=1 / trace=True); safe no-op
    otherwise. Mirrors trn_agent_boot.trn_boot step 6.
    """
    try:
        if "antenv.axon_hooks" in sys.modules:
            return
        import antenv

        mod = types.ModuleType("antenv.axon_hooks")
        state = {"hook": None}
        mod.set_axon_ntff_profile_hook = lambda h: state.__setitem__("hook", h)
        mod.get_axon_ntff_profile_hook = lambda: state["hook"]
        sys.modules["antenv.axon_hooks"] = mod
        antenv.axon_hooks = mod
        try:
            from trn_agent_boot.trn_boot import _ntff_profile_via_ctypes

            mod.set_axon_ntff_profile_hook(
                _ntff_profile_via_ctypes("/opt/axon/libaxon_pjrt.so")
            )
        except Exception:
            pass
    except Exception:
        pass


def _routing(top_k_indices, top_k_weights):
    """Per-expert slot lists (ascending flat order == Switch dispatch pos),
    clipped at CAP exactly like the reference's capacity drop."""
    e_flat = np.asarray(top_k_indices).reshape(-1).astype(np.int32)
    w_flat = np.asarray(top_k_weights).reshape(-1).astype(np.float32)
    tok = np.arange(T * TOPK, dtype=np.int32) // TOPK
    order = np.argsort(e_flat, kind="stable")
    sorted_e = e_flat[order]
    starts = np.searchsorted(sorted_e, np.arange(E + 1))
    slots_per_e = [order[starts[e] : starts[e + 1]][:CAP] for e in range(E)]
    return e_flat, w_flat, tok, slots_per_e


_prog_cache = {}


def _build_program(m_pads):
    """One SPMD program: per-core grouped SwiGLU MLP over EPC experts,
    position j padded to m_pads[j] slots."""
    import concourse.bacc as bacc
    import concourse.mybir as mybir
    from concourse.tile import TileContext

    f32 = mybir.dt.float32
    f16 = mybir.dt.float16
    slots = int(sum(m_pads))
    offs = [0]
    for m in m_pads:
        offs.append(offs[-1] + int(m))

    nc = bacc.Bacc("TRN2", target_bir_lowering=False, debug=False,
                   num_devices=NCORES)
    # Host layouts (all fp16 except wsc):
    #   xdT[p, HT*off_j + ht*m_j + s]          = xd[off_j+s, ht*128+p]
    #   w1t[j, p, it*1024 + ht*256 + g*128 + o'] = gate_up[e_j,
    #         g*I + it*128 + o', ht*128 + p]     (g=0 gate, g=1 up)
    #   w2t[j, p, it*H + h]                    = down[e_j, h, it*128+p]
    #   y[p, HT*off_j + ht*m_j + s]            = y_out[off_j+s, ht*128+p]
    xdT_d = nc.declare_dram_parameter("xdT", [128, HT * slots], f16,
                                      isOutput=False)
    # xw0 = [xd(e0) | w1(e0)] packed so the kernel's first DMA is one
    # large-elem transfer covering everything the first two it-groups need.
    xw0_d = nc.declare_dram_parameter(
        "xw0", [128, HT * int(m_pads[0]) + IT * 1024], f16, isOutput=False)
    w1t_d = nc.declare_dram_parameter("w1t", [EPC, 128, IT * 2 * 512], f16,
                                      isOutput=False)
    w2t_d = nc.declare_dram_parameter("w2t", [EPC, 128, IT * H], f16,
                                      isOutput=False)
    wsc_d = nc.declare_dram_parameter("wsc", [1, slots], f32,
                                      isOutput=False)
    y_d = nc.declare_dram_parameter("y", [128, HT * slots], f16,
                                    isOutput=True)

    W1C = IT * 1024  # 8192 cols per expert in w1t
    m0 = int(m_pads[0])

    with TileContext(nc) as tc:
        with (
            tc.tile_pool(name="xd", bufs=1) as xdp,
            tc.tile_pool(name="w1", bufs=1) as w1p,
            tc.tile_pool(name="w2", bufs=1) as w2p,
            tc.tile_pool(name="act", bufs=1) as actp,
            tc.tile_pool(name="ps1", bufs=3, space="PSUM") as ps1p,
            tc.tile_pool(name="ps2", bufs=2, space="PSUM") as ps2p,
            tc.tile_pool(name="outp", bufs=2) as outp,
            tc.tile_pool(name="misc", bufs=1) as miscp,
        ):
            # ---- front-loaded DMA issues ----
            # The HWDGE rings run ~400 GB/s on large-elem transfers but only
            # ~150-250 GB/s on small ones, and the two rings share the HBM
            # port. So: keep transfers large, put the whole load stream on
            # the SP ring in exact consumption order (FIFO = schedule), and
            # leave the ACT ring to the dep-bound stores. The PE is the
            # bottleneck from e0's second half onward.
            #   SP ring: [xd0|w1(e0) it0-1], w1(e0) it2-7 pairs, w2(e0),
            #            xd(e1-e3), then per expert j>=1: w1 halves, w2.
            #   ACT ring: output stores only.
            w2t = [w2p.tile([128, IT * H], f16, tag=f"w2e{j}",
                            name=f"w2e{j}") for j in range(EPC)]
            # First DMA: one large-elem transfer holding xd(e0) plus w1(e0)
            # it-groups 0-1 — everything the first two psum groups need.
            XB = HT * m0
            xw0a = w1p.tile([128, XB + 2048], f16, tag="xw0a", name="xw0a")
            nc.sync.dma_start(out=xw0a[:], in_=xw0_d[:, 0 : XB + 2048])
            w1e0 = [xw0a]
            for p in range(1, 4):
                t = w1p.tile([128, 2048], f16, tag=f"w1e0p{p}",
                             name=f"w1e0p{p}")
                nc.sync.dma_start(
                    out=t[:],
                    in_=xw0_d[:, XB + p * 2048 : XB + (p + 1) * 2048])
                w1e0.append(t)
            nc.sync.dma_start(out=w2t[0][:], in_=w2t_d[0])
            xdr = xdp.tile([128, HT * (slots - m0)], f16, tag="xdr",
                           name="xdr")
            nc.sync.dma_start(out=xdr[:], in_=xdT_d[:, HT * m0 :])

            # Warm-up: matmuls on a zeroed tile ramp the PE p-state
            # (0.65 -> 2.4 GHz takes ~3us of continuous busy) while the
            # first real loads are in flight; they also keep the PE from
            # going idle (which would reset the ramp) until data lands.
            # The memset comes before gpsimd's wsc issue so warm-up can
            # begin as early as possible.
            zt = actp.tile([128, m0], f16, tag="zt", name="zt")
            nc.gpsimd.memset(zt[:], 0.0)

            # software DGE: routing weights (tiny)
            wsc_t = miscp.tile([1, slots], f32, tag="wsc")
            nc.gpsimd.dma_start(out=wsc_t[:], in_=wsc_d[:])
            for _ in range(17):
                pw = ps2p.tile([128, m0], f32, tag="ps2", name="ps2")
                nc.tensor.matmul(pw[:], zt[:, 0:128], zt[:],
                                 start=True, stop=True)

            # SP ring, consumption order for experts 1-3
            w1f = [None] * EPC  # per expert j>=1: [half0, half1] tiles
            for j in range(1, EPC):
                halves = []
                for h in range(2):
                    ht_ = w1p.tile([128, W1C // 2], f16, tag=f"w1f{h}",
                                   name=f"w1f{h}", bufs=2)
                    nc.sync.dma_start(
                        out=ht_[:],
                        in_=w1t_d[j, :, h * (W1C // 2) : (h + 1) * (W1C // 2)])
                    halves.append(ht_)
                w1f[j] = halves
                nc.sync.dma_start(out=w2t[j][:], in_=w2t_d[j])

            # ---- per-expert compute ----
            pending_store = None  # (ot tile, dram col base, m)
            for j in range(EPC):
                m_pad = int(m_pads[j])
                xbase = HT * offs[j]

                def xd_sl(ht, j=j, m_pad=m_pad):
                    if j == 0:
                        return xw0a[:, ht * m_pad : (ht + 1) * m_pad]
                    b = HT * (offs[j] - m0)
                    return xdr[:, b + ht * m_pad : b + (ht + 1) * m_pad]

                def w1_sl(it, ht, g, j=j):
                    # columns it*1024 + ht*256 + g*128 .. +128
                    if j == 0:
                        t, rem = w1e0[it // 2], it % 2
                        base = XB if it < 2 else 0
                    else:
                        half, rem = divmod(it, IT // 2)
                        t = w1f[j][half]
                        base = 0
                    c = base + rem * 1024 + ht * 256 + g * 128
                    return t[:, c : c + 128]

                # mm1 + silu*up, it-major (consumes w1 pieces in order).
                # w2 loads are interleaved into the silu stream: issued
                # early enough to land before their mm2, late enough not to
                # starve the w1 piece stream or delay a critical silu.
                acts = []
                for it in range(IT):
                    pg = ps1p.tile([128, m_pad], f32, tag="pg", name="pg")
                    pu = ps1p.tile([128, m_pad], f32, tag="pu", name="pu")
                    for ht in range(HT):
                        nc.tensor.matmul(pg[:], w1_sl(it, ht, 0), xd_sl(ht),
                                         start=(ht == 0), stop=(ht == HT - 1))
                    for ht in range(HT):
                        nc.tensor.matmul(pu[:], w1_sl(it, ht, 1), xd_sl(ht),
                                         start=(ht == 0), stop=(ht == HT - 1))
                    sg = actp.tile([128, m_pad], f32, tag="sg", name="sg",
                                   bufs=3)
                    nc.scalar.activation(
                        sg[:], pg[:], mybir.ActivationFunctionType.Silu)
                    a = actp.tile([128, m_pad], f16, tag=f"a{it}",
                                  name=f"a{it}", bufs=2)
                    nc.vector.tensor_mul(a[:], sg[:], pu[:])
                    acts.append(a)

                # previous expert's output store, after this expert's silus
                if pending_store is not None:
                    ot_p, base_p, mlen_p = pending_store
                    nc.scalar.dma_start(
                        out=y_d[:, base_p : base_p + HT * mlen_p],
                        in_=ot_p[:])
                    pending_store = None

                # routing-weight row broadcast for the column scale
                wrow = miscp.tile([128, m_pad], f32, tag="wrow", name="wrow",
                                  bufs=2)
                nc.gpsimd.partition_broadcast(
                    wrow[:], wsc_t[0:1, offs[j] : offs[j] + m_pad])

                # mm2 + column scale -> fp16 out tile(s). The last expert
                # uses one tile per h-chunk so each store depends only on
                # its own chunk's scale mul (tile-granular deps).
                last = j == EPC - 1
                if not last:
                    ot = outp.tile([128, HT * m_pad], f16, tag="ot",
                                   name="ot")
                for ht2 in range(HT):
                    ps2 = ps2p.tile([128, m_pad], f32, tag="ps2", name="ps2")
                    for it in range(IT):
                        c = ht2 * I + it * 128
                        nc.tensor.matmul(
                            ps2[:], w2t[j][:, c : c + 128], acts[it][:],
                            start=(it == 0), stop=(it == IT - 1))
                    if last:
                        otc = outp.tile([128, m_pad], f16, tag=f"ot3{ht2}",
                                        name=f"ot3{ht2}")
                        nc.vector.tensor_mul(otc[:], ps2[:], wrow[:])
                        # tail: store each h-chunk as soon as it's scaled
                        nc.scalar.dma_start(
                            out=y_d[:, xbase + ht2 * m_pad :
                                    xbase + (ht2 + 1) * m_pad],
                            in_=otc[:])
                    else:
                        nc.vector.tensor_mul(
                            ot[:, ht2 * m_pad : (ht2 + 1) * m_pad], ps2[:],
                            wrow[:])
                if not last:
                    pending_store = (ot, xbase, m_pad)

    nc.finalize()
    return nc


def kernel(hidden_states, top_k_indices, top_k_weights, gate_up_proj,
           down_proj):
    global LAST_RESULTS
    _ensure_profile_hook()
    from concourse.bass_utils import run_bass_kernel_spmd

    hs = np.ascontiguousarray(np.asarray(hidden_states, dtype=np.float32))
    gup = np.asarray(gate_up_proj, dtype=np.float32)
    dwn = np.asarray(down_proj, dtype=np.float32)

    e_flat, w_flat, tok, slots_per_e = _routing(top_k_indices, top_k_weights)
    counts = np.array([len(s) for s in slots_per_e])
    # Load-balance: sort experts by routed count and deal them out in
    # rounds of NCORES — position j on every core handles one expert from
    # round j, so the per-position compile-time pad (the round max) is the
    # smallest achievable (matches the order-statistics lower bound).
    sorted_eids = np.argsort(-counts, kind="stable")
    assign = sorted_eids.reshape(EPC, NCORES)  # [position, core]
    # Pads are 2-aligned (even) — every padded column costs 96 PE cycles,
    # so keep the rounding as tight as the fp16 layouts allow.
    m_pads = tuple(
        int(min(CAP, max(128, ((int(counts[assign[j]].max()) + 1) // 2) * 2)))
        for j in range(EPC))
    offs = [0]
    for m in m_pads:
        offs.append(offs[-1] + m)
    slots = offs[-1]

    if m_pads not in _prog_cache:
        _prog_cache[m_pads] = _build_program(m_pads)
    nc = _prog_cache[m_pads]

    in_maps = []
    core_exps = []
    for c in range(NCORES):
        exps = [int(assign[j, c]) for j in range(EPC)]
        core_exps.append(exps)
        xd = np.zeros((slots, H), np.float32)
        wsc = np.zeros((1, slots), np.float32)
        for j, e in enumerate(exps):
            sl = slots_per_e[e]
            xd[offs[j] : offs[j] + len(sl)] = hs[tok[sl]]
            wsc[0, offs[j] : offs[j] + len(sl)] = w_flat[sl]
        # xdT[p, HT*off_j + ht*m_j + s] = xd[off_j + s, ht*128 + p]
        parts = []
        for j in range(EPC):
            blk = xd[offs[j] : offs[j + 1]]  # [m_j, H]
            parts.append(
                blk.reshape(m_pads[j], HT, 128).transpose(2, 1, 0)
                .reshape(128, HT * m_pads[j]))
        xdT = np.ascontiguousarray(np.concatenate(parts, axis=1)
                                   .astype(np.float16))
        # w1t[j, p, it*1024 + ht*256 + g*128 + o'] =
        #   gate_up[e_j, g*I + it*128 + o', ht*128 + p]
        w1t = np.ascontiguousarray(
            gup[exps].reshape(EPC, 2, IT, 128, HT, 128)
            .transpose(0, 5, 2, 4, 1, 3)  # [j, p, it, ht, g, o']
            .astype(np.float16)).reshape(EPC, 128, IT * 1024)
        # w2t[j, p, ht2*1024 + it*128 + h''] = down[e_j, ht2*128+h'', it*128+p]
        w2t = np.ascontiguousarray(
            dwn[exps].reshape(EPC, HT, 128, IT, 128)
            .transpose(0, 4, 1, 3, 2)
            .astype(np.float16)).reshape(EPC, 128, IT * H)
        xw0 = np.ascontiguousarray(
            np.concatenate([xdT[:, : HT * m_pads[0]], w1t[0]], axis=1))
        in_maps.append({"xdT": xdT, "xw0": xw0, "w1t": w1t, "w2t": w2t,
                        "wsc": wsc})

    res = run_bass_kernel_spmd(nc, in_maps, list(range(NCORES)))
    LAST_RESULTS = res

    # Combine: scatter per-slot outputs back to flat (token, k) slots and
    # reduce over the top-k axis and cores.
    y_tk = np.zeros((T * TOPK, H), np.float32)
    for c in range(NCORES):
        yc = res.results[c]["y"]  # [128, HT*slots] fp16; y^T[h, s] blocks
        for j, e in enumerate(core_exps[c]):
            sl = slots_per_e[e]
            blk = (yc[:, HT * offs[j] : HT * offs[j + 1]]
                   .reshape(128, HT, m_pads[j]).astype(np.float32))
            # y[s, h] with h = ht*128 + p
            y_tk[sl] = blk.transpose(2, 1, 0).reshape(m_pads[j], H)[: len(sl)]
    return y_tk.reshape(T, TOPK, H).sum(axis=1)
